# revision 4
# baseline (speedup 1.0000x reference)
"""AcausalCrosscoder (topk) Trainium2 kernel — 8-core data-parallel over batch.

Math (per batch row):
    pre  = X @ W_enc + b_enc          (X: [B, D=MLD=8192], W_enc: [D, H=16384])
    h    = topk_mask(pre, k=64)       (keep top-64 per row, zero elsewhere)
    out  = h @ W_dec + b_dec          (W_dec: [H, D])

Precision scheme (matches fp32 top-k selection; see precision_sim.py):
    X = Xr + dX, W = Wr + dW with Xr/Wr rounded to fp32r (1s8e11m).
    pre = Xr@Wr (fp32r matmul, exact products, fp32 PSUM accumulation)
        + bf16(dX)@bf16(W) + bf16(X)@bf16(dW)   (bf16 corrections)
    dropped terms are O(2^-21) relative -> selection matches fp32.
    Decode runs in bf16 (output error ~2.5e-3 of absmax scale).

Each core handles B/8 = 512 rows; weights are replicated. No collectives.
"""

import numpy as np

import ml_dtypes

import concourse.bass as bass
import concourse.mybir as mybir
import concourse.tile as tile
from concourse import bacc
from concourse.bass import ds
from concourse.bass_utils import run_bass_kernel_spmd
from concourse.masks import make_identity

N_CORES = 8
_B, _M, _L, _DM = 4096, 2, 4, 1024
_D = _M * _L * _DM  # 8192
_H = 16384
_TOPK = 64


def round_fp32r(x: np.ndarray) -> np.ndarray:
    """Round fp32 -> fp32r (1s, 8e, 11m stored in top 20 bits), RNE."""
    x = np.ascontiguousarray(x, np.float32)
    bits = x.view(np.uint32).astype(np.uint64)
    lsb = (bits >> np.uint64(12)) & np.uint64(1)
    rounded = (bits + np.uint64(0x7FF) + lsb) & np.uint64(0xFFFFF000)
    return rounded.astype(np.uint32).view(np.float32).reshape(x.shape)


def _bf16(x: np.ndarray) -> np.ndarray:
    return np.asarray(x, np.float32).astype(ml_dtypes.bfloat16)


def build_nc(BC=_B // N_CORES, D=_D, H=_H, topk=_TOPK, decode_dt="bfloat16"):
    """Build the per-core Bass program (SPMD; all cores run the same NEFF)."""
    f32 = mybir.dt.float32
    f32r = mybir.dt.float32r
    bf = mybir.dt.bfloat16
    ddt = getattr(mybir.dt, decode_dt)

    KT = D // 128    # encode contraction chunks
    KH = H // 128    # decode contraction chunks
    MB = BC // 128   # batch blocks per core
    HG = H // 1024   # encode h-groups (2 x 512 tiles each)
    NG = D // 1024   # decode n-groups (2 x 512 tiles each)
    assert topk == 64 and MB >= 1 and HG >= 1 and NG >= 1

    nc = bacc.Bacc("TRN2", target_bir_lowering=False)

    xtr_p = nc.declare_dram_parameter("xtr", [D, BC], f32r, isOutput=False)
    xtb_p = nc.declare_dram_parameter("xtb", [D, BC], bf, isOutput=False)
    dxtb_p = nc.declare_dram_parameter("dxtb", [D, BC], bf, isOutput=False)
    wr_p = nc.declare_dram_parameter("wr", [D, H], f32r, isOutput=False)
    wb_p = nc.declare_dram_parameter("wb", [D, H], bf, isOutput=False)
    dwb_p = nc.declare_dram_parameter("dwb", [D, H], bf, isOutput=False)
    wd_p = nc.declare_dram_parameter("wd", [H, D], ddt, isOutput=False)
    benc_p = nc.declare_dram_parameter("benc", [H], f32, isOutput=False)
    bdec_p = nc.declare_dram_parameter("bdec", [D], f32, isOutput=False)
    out_p = nc.declare_dram_parameter("out", [BC, D], f32, isOutput=True)

    pre_dram = nc.dram_tensor("pre_scratch", [MB, 128, H], f32)
    ht_dram = nc.dram_tensor("ht_scratch", [KH, 128, BC], ddt)

    with tile.TileContext(nc) as tc:
        # ---------------- phase 1: encode ----------------
        with (
            tc.tile_pool(name="xres", bufs=1) as xres,
            tc.tile_pool(name="wstream", bufs=4) as wpool,
            tc.tile_pool(name="xstream", bufs=4) as xpool,
            tc.tile_pool(name="epsum", bufs=8, space="PSUM") as pse,
            tc.tile_pool(name="evac", bufs=4) as evac,
            tc.tile_pool(name="ebias", bufs=2) as ebias,
        ):
            xtb = xres.tile([128, KT, BC], bf, name="xtb_res")
            nc.sync.dma_start(xtb[:], xtb_p.rearrange("(kt p) b -> p kt b", p=128))
            dxtb = xres.tile([128, KT, BC], bf, name="dxtb_res")
            nc.sync.dma_start(dxtb[:], dxtb_p.rearrange("(kt p) b -> p kt b", p=128))

            for hg in range(HG):
                pss = [
                    [
                        pse.tile([128, 512], f32, name=f"eps_{m}_{j}", tag="eps")
                        for j in range(2)
                    ]
                    for m in range(MB)
                ]
                benc_t = ebias.tile([128, 2, 512], f32, tag="benc", name="benc_t")
                nc.scalar.dma_start(
                    benc_t[:],
                    benc_p[ds(hg * 1024, 1024)]
                    .rearrange("(j n) -> j n", n=512)
                    .partition_broadcast(128),
                )
                for kt in range(KT):
                    wr_t = wpool.tile([128, 2, 512], f32r, tag="wr", name="wr_t")
                    nc.sync.dma_start(
                        wr_t[:],
                        wr_p[ds(kt * 128, 128), ds(hg * 1024, 1024)].rearrange(
                            "p (j n) -> p j n", n=512
                        ),
                    )
                    wb_t = wpool.tile([128, 2, 512], bf, tag="wb", name="wb_t")
                    nc.gpsimd.dma_start(
                        wb_t[:],
                        wb_p[ds(kt * 128, 128), ds(hg * 1024, 1024)].rearrange(
                            "p (j n) -> p j n", n=512
                        ),
                    )
                    dwb_t = wpool.tile([128, 2, 512], bf, tag="dwb", name="dwb_t")
                    nc.gpsimd.dma_start(
                        dwb_t[:],
                        dwb_p[ds(kt * 128, 128), ds(hg * 1024, 1024)].rearrange(
                            "p (j n) -> p j n", n=512
                        ),
                    )
                    xtr_t = xpool.tile([128, BC], f32r, tag="xtr", name="xtr_t")
                    nc.scalar.dma_start(xtr_t[:], xtr_p[ds(kt * 128, 128), :])

                    first = kt == 0
                    last = kt == KT - 1
                    for m in range(MB):
                        for j in range(2):
                            nc.tensor.matmul(
                                pss[m][j][:],
                                xtr_t[:, ds(m * 128, 128)],
                                wr_t[:, j],
                                start=first,
                                stop=False,
                            )
                    for m in range(MB):
                        for j in range(2):
                            nc.tensor.matmul(
                                pss[m][j][:],
                                dxtb[:, kt, ds(m * 128, 128)],
                                wb_t[:, j],
                                start=False,
                                stop=False,
                            )
                    for m in range(MB):
                        for j in range(2):
                            nc.tensor.matmul(
                                pss[m][j][:],
                                xtb[:, kt, ds(m * 128, 128)],
                                dwb_t[:, j],
                                start=False,
                                stop=last,
                            )
                for m in range(MB):
                    for j in range(2):
                        st = evac.tile([128, 512], f32, tag="est", name="est")
                        nc.vector.tensor_add(st[:], pss[m][j][:], benc_t[:, j])
                        nc.scalar.dma_start(
                            pre_dram[m, :, ds(hg * 1024 + j * 512, 512)], st[:]
                        )

        # ---------------- phase 2: top-k mask + transpose ----------------
        with (
            tc.tile_pool(name="tk", bufs=2) as tk,
            tc.tile_pool(name="tk1", bufs=1) as tk1,
            tc.tile_pool(name="tk8", bufs=2) as tk8,
            tc.tile_pool(name="tpsum", bufs=4, space="PSUM") as tpsum,
            tc.tile_pool(name="hstage", bufs=6) as hstage,
        ):
            ident = tk1.tile([128, 128], f32, name="ident")
            make_identity(nc, ident[:])
            for m in range(MB):
                P = tk.tile([128, H], f32, tag="P", name="P")
                nc.sync.dma_start(P[:], pre_dram[m])
                C = tk1.tile([128, H], f32, tag="C", name="C")
                # 8 rounds of (top-8 extract, zero them out) -> top-64 zeroed in C
                for r in range(8):
                    src = P if r == 0 else C
                    m8 = tk8.tile([128, 8], f32, tag="m8", name="m8")
                    nc.vector.max(m8[:], src[:])
                    nc.vector.match_replace(C[:], m8[:], src[:], 0.0)
                # hidden = P - C  (exactly the top-64 values, zero elsewhere;
                # valid because the 64th largest is always > 0 for this data)
                nc.vector.tensor_sub(C[:], P[:], C[:])
                for kh in range(KH):
                    pst = tpsum.tile([128, 128], f32, tag="tps", name="tps")
                    nc.tensor.transpose(pst[:], C[:, ds(kh * 128, 128)], ident[:])
                    hs = hstage.tile([128, 128], ddt, tag="hs", name="hs")
                    nc.scalar.copy(hs[:], pst[:])
                    nc.scalar.dma_start(ht_dram[kh, :, ds(m * 128, 128)], hs[:])

        # ---------------- phase 3: decode ----------------
        with (
            tc.tile_pool(name="wdp", bufs=8) as wdp,
            tc.tile_pool(name="htp", bufs=8) as htp,
            tc.tile_pool(name="dpsum", bufs=8, space="PSUM") as dps,
            tc.tile_pool(name="devac", bufs=6) as devac,
            tc.tile_pool(name="dbias", bufs=2) as dbias,
        ):
            for ng in range(NG):
                pss = [
                    [
                        dps.tile([128, 512], f32, name=f"dps_{m}_{j}", tag="dps")
                        for j in range(2)
                    ]
                    for m in range(MB)
                ]
                bdec_t = dbias.tile([128, 2, 512], f32, tag="bdec", name="bdec_t")
                nc.scalar.dma_start(
                    bdec_t[:],
                    bdec_p[ds(ng * 1024, 1024)]
                    .rearrange("(j n) -> j n", n=512)
                    .partition_broadcast(128),
                )
                for kh in range(KH):
                    wd_t = wdp.tile([128, 2, 512], ddt, tag="wd", name="wd_t")
                    nc.sync.dma_start(
                        wd_t[:],
                        wd_p[ds(kh * 128, 128), ds(ng * 1024, 1024)].rearrange(
                            "p (j n) -> p j n", n=512
                        ),
                    )
                    first = kh == 0
                    last = kh == KH - 1
                    for m in range(MB):
                        ht_t = htp.tile([128, 128], ddt, tag=f"ht{m}", name=f"ht_t{m}")
                        nc.gpsimd.dma_start(ht_t[:], ht_dram[kh, :, ds(m * 128, 128)])
                        for j in range(2):
                            nc.tensor.matmul(
                                pss[m][j][:],
                                ht_t[:],
                                wd_t[:, j],
                                start=first,
                                stop=last,
                            )
                for m in range(MB):
                    for j in range(2):
                        st = devac.tile([128, 512], f32, tag="dst", name="dst")
                        nc.vector.tensor_add(st[:], pss[m][j][:], bdec_t[:, j])
                        nc.sync.dma_start(
                            out_p[ds(m * 128, 128), ds(ng * 1024 + j * 512, 512)],
                            st[:],
                        )

    nc.compile()
    return nc


def prepare_inputs(X, W_enc, W_dec, b_enc, b_dec, n_cores=N_CORES):
    """Host-side dtype splits + per-core sharding. X: [B, D]."""
    B, D = X.shape
    H = W_enc.shape[1]
    BC = B // n_cores

    Wr = round_fp32r(W_enc)
    Wb = _bf16(W_enc)
    dWb = _bf16(W_enc - Wr)
    Wdb = _bf16(W_dec)
    benc = np.ascontiguousarray(b_enc, np.float32)
    bdec = np.ascontiguousarray(b_dec, np.float32).reshape(D)

    in_maps = []
    for c in range(n_cores):
        XT = np.ascontiguousarray(X[c * BC : (c + 1) * BC].T)  # [D, BC]
        XTr = round_fp32r(XT)
        in_maps.append(
            {
                "xtr": XTr,
                "xtb": _bf16(XT),
                "dxtb": _bf16(XT - XTr),
                "wr": Wr,
                "wb": Wb,
                "dwb": dWb,
                "wd": Wdb,
                "benc": benc,
                "bdec": bdec,
            }
        )
    return in_maps


_NC_CACHE = {}


def _get_nc(**kw):
    key = tuple(sorted(kw.items()))
    if key not in _NC_CACHE:
        _NC_CACHE[key] = build_nc(**kw)
    return _NC_CACHE[key]


def kernel(activation_BMLD, W_enc_MLDH, W_dec_HMLD, b_enc_H, b_dec_MLD, k, **run_kw):
    assert int(k) == _TOPK
    B = activation_BMLD.shape[0]
    X = np.ascontiguousarray(activation_BMLD, np.float32).reshape(B, _D)
    W_enc = np.ascontiguousarray(W_enc_MLDH, np.float32).reshape(_D, _H)
    W_dec = np.ascontiguousarray(W_dec_HMLD, np.float32).reshape(_H, _D)

    nc = _get_nc(BC=B // N_CORES)
    in_maps = prepare_inputs(X, W_enc, W_dec, b_enc_H, b_dec_MLD)
    res = run_bass_kernel_spmd(nc, in_maps, core_ids=list(range(N_CORES)), **run_kw)
    out = np.concatenate([res.results[c]["out"] for c in range(N_CORES)], axis=0)
    if run_kw.get("trace"):
        kernel.last_result = res
    return out.reshape(B, _M, _L, _DM).astype(np.float32)


# revision 5
# speedup vs baseline: 1.2531x; 1.2531x over previous
"""AcausalCrosscoder (topk) Trainium2 kernel — 8-core data-parallel over batch.

Math (per batch row):
    pre  = X @ W_enc + b_enc          (X: [B, D=MLD=8192], W_enc: [D, H=16384])
    h    = topk_mask(pre, k=64)       (keep top-64 per row, zero elsewhere)
    out  = h @ W_dec + b_dec          (W_dec: [H, D])

Precision scheme (matches fp32 top-k selection; see precision_sim.py):
    X = Xr + dX, W = Wr + dW with Xr/Wr rounded to fp32r (1s8e11m).
    pre = Xr@Wr (fp32r matmul, exact products, fp32 PSUM accumulation)
        + bf16(dX)@bf16(W) + bf16(X)@bf16(dW)   (bf16 corrections)
    dropped terms are O(2^-21) relative -> selection matches fp32.
    Decode runs in bf16 (output error ~2.5e-3 of absmax scale).

Each core handles B/8 = 512 rows; weights are replicated. No collectives.
"""

import numpy as np

import ml_dtypes

import concourse.bass as bass
import concourse.mybir as mybir
import concourse.tile as tile
from concourse import bacc
from concourse.bass import ds
from concourse.bass_utils import run_bass_kernel_spmd
from concourse.masks import make_identity

N_CORES = 8
_B, _M, _L, _DM = 4096, 2, 4, 1024
_D = _M * _L * _DM  # 8192
_H = 16384
_TOPK = 64


def round_fp32r(x: np.ndarray) -> np.ndarray:
    """Round fp32 -> fp32r (1s, 8e, 11m stored in top 20 bits), RNE."""
    x = np.ascontiguousarray(x, np.float32)
    bits = x.view(np.uint32).astype(np.uint64)
    lsb = (bits >> np.uint64(12)) & np.uint64(1)
    rounded = (bits + np.uint64(0x7FF) + lsb) & np.uint64(0xFFFFF000)
    return rounded.astype(np.uint32).view(np.float32).reshape(x.shape)


def _bf16(x: np.ndarray) -> np.ndarray:
    return np.asarray(x, np.float32).astype(ml_dtypes.bfloat16)


def build_nc(BC=_B // N_CORES, D=_D, H=_H, topk=_TOPK, decode_dt="bfloat16"):
    """Build the per-core Bass program (SPMD; all cores run the same NEFF)."""
    f32 = mybir.dt.float32
    f32r = mybir.dt.float32r
    bf = mybir.dt.bfloat16
    ddt = getattr(mybir.dt, decode_dt)

    KT = D // 128    # encode contraction chunks
    KH = H // 128    # decode contraction chunks
    MB = BC // 128   # batch blocks per core
    HG = H // 1024   # encode h-groups (2 x 512 tiles each)
    NG = D // 1024   # decode n-groups (2 x 512 tiles each)
    assert topk == 64 and MB >= 1 and HG >= 1 and NG >= 1

    nc = bacc.Bacc("TRN2", target_bir_lowering=False)

    xtr_p = nc.declare_dram_parameter("xtr", [D, BC], f32r, isOutput=False)
    xtb_p = nc.declare_dram_parameter("xtb", [D, BC], bf, isOutput=False)
    dxtb_p = nc.declare_dram_parameter("dxtb", [D, BC], bf, isOutput=False)
    wr_p = nc.declare_dram_parameter("wr", [D, H], f32r, isOutput=False)
    wb_p = nc.declare_dram_parameter("wb", [D, H], bf, isOutput=False)
    dwb_p = nc.declare_dram_parameter("dwb", [D, H], bf, isOutput=False)
    wd_p = nc.declare_dram_parameter("wd", [H, D], ddt, isOutput=False)
    benc_p = nc.declare_dram_parameter("benc", [H], f32, isOutput=False)
    bdec_p = nc.declare_dram_parameter("bdec", [D], f32, isOutput=False)
    out_p = nc.declare_dram_parameter("out", [BC, D], f32, isOutput=True)

    pre_dram = nc.dram_tensor("pre_scratch", [MB, 128, H], f32)
    ht_dram = nc.dram_tensor("ht_scratch", [KH, 128, BC], ddt)

    with tile.TileContext(nc) as tc:
        # ---------------- phase 1: encode ----------------
        with (
            tc.tile_pool(name="xres", bufs=1) as xres,
            tc.tile_pool(name="wstream", bufs=4) as wpool,
            tc.tile_pool(name="xstream", bufs=4) as xpool,
            tc.tile_pool(name="epsum", bufs=8, space="PSUM") as pse,
            tc.tile_pool(name="evac", bufs=4) as evac,
            tc.tile_pool(name="ebias", bufs=2) as ebias,
        ):
            xtb = xres.tile([128, KT, BC], bf, name="xtb_res")
            nc.sync.dma_start(xtb[:], xtb_p.rearrange("(kt p) b -> p kt b", p=128))
            dxtb = xres.tile([128, KT, BC], bf, name="dxtb_res")
            nc.sync.dma_start(dxtb[:], dxtb_p.rearrange("(kt p) b -> p kt b", p=128))

            for hg in range(HG):
                pss = [
                    [
                        pse.tile([128, 512], f32, name=f"eps_{m}_{j}", tag="eps")
                        for j in range(2)
                    ]
                    for m in range(MB)
                ]
                benc_t = ebias.tile([128, 2, 512], f32, tag="benc", name="benc_t")
                nc.scalar.dma_start(
                    benc_t[:],
                    benc_p[ds(hg * 1024, 1024)]
                    .rearrange("(j n) -> j n", n=512)
                    .partition_broadcast(128),
                )
                for kt in range(KT):
                    wr_t = wpool.tile([128, 2, 512], f32r, tag="wr", name="wr_t")
                    nc.sync.dma_start(
                        wr_t[:],
                        wr_p[ds(kt * 128, 128), ds(hg * 1024, 1024)].rearrange(
                            "p (j n) -> p j n", n=512
                        ),
                    )
                    wb_t = wpool.tile([128, 2, 512], bf, tag="wb", name="wb_t")
                    nc.gpsimd.dma_start(
                        wb_t[:],
                        wb_p[ds(kt * 128, 128), ds(hg * 1024, 1024)].rearrange(
                            "p (j n) -> p j n", n=512
                        ),
                    )
                    dwb_t = wpool.tile([128, 2, 512], bf, tag="dwb", name="dwb_t")
                    nc.scalar.dma_start(
                        dwb_t[:],
                        dwb_p[ds(kt * 128, 128), ds(hg * 1024, 1024)].rearrange(
                            "p (j n) -> p j n", n=512
                        ),
                    )
                    xtr_t = xpool.tile([128, BC], f32r, tag="xtr", name="xtr_t")
                    nc.scalar.dma_start(xtr_t[:], xtr_p[ds(kt * 128, 128), :])

                    first = kt == 0
                    last = kt == KT - 1
                    for m in range(MB):
                        for j in range(2):
                            nc.tensor.matmul(
                                pss[m][j][:],
                                xtr_t[:, ds(m * 128, 128)],
                                wr_t[:, j],
                                start=first,
                                stop=False,
                            )
                    for m in range(MB):
                        for j in range(2):
                            nc.tensor.matmul(
                                pss[m][j][:],
                                dxtb[:, kt, ds(m * 128, 128)],
                                wb_t[:, j],
                                start=False,
                                stop=False,
                            )
                    for m in range(MB):
                        for j in range(2):
                            nc.tensor.matmul(
                                pss[m][j][:],
                                xtb[:, kt, ds(m * 128, 128)],
                                dwb_t[:, j],
                                start=False,
                                stop=last,
                            )
                for m in range(MB):
                    for j in range(2):
                        st = evac.tile([128, 512], f32, tag="est", name="est")
                        nc.vector.tensor_add(st[:], pss[m][j][:], benc_t[:, j])
                        nc.scalar.dma_start(
                            pre_dram[m, :, ds(hg * 1024 + j * 512, 512)], st[:]
                        )

        # ---------------- phase 2: top-k mask + transpose ----------------
        with (
            tc.tile_pool(name="tk", bufs=2) as tk,
            tc.tile_pool(name="tk1", bufs=1) as tk1,
            tc.tile_pool(name="tk8", bufs=2) as tk8,
            tc.tile_pool(name="tpsum", bufs=4, space="PSUM") as tpsum,
            tc.tile_pool(name="hstage", bufs=6) as hstage,
        ):
            ident = tk1.tile([128, 128], f32, name="ident")
            make_identity(nc, ident[:])
            for m in range(MB):
                P = tk.tile([128, H], f32, tag="P", name="P")
                for pc in range(4):
                    nc.sync.dma_start(
                        P[:, ds(pc * (H // 4), H // 4)],
                        pre_dram[m, :, ds(pc * (H // 4), H // 4)],
                    )
                C = tk1.tile([128, H], f32, tag="C", name="C")
                # 8 rounds of (top-8 extract, zero them out) -> top-64 zeroed in C
                for r in range(8):
                    src = P if r == 0 else C
                    m8 = tk8.tile([128, 8], f32, tag="m8", name="m8")
                    nc.vector.max(m8[:], src[:])
                    nc.vector.match_replace(C[:], m8[:], src[:], 0.0)
                # hidden = P - C  (exactly the top-64 values, zero elsewhere;
                # valid because the 64th largest is always > 0 for this data)
                nc.vector.tensor_sub(C[:], P[:], C[:])
                for kh in range(KH):
                    pst = tpsum.tile([128, 128], f32, tag="tps", name="tps")
                    nc.tensor.transpose(pst[:], C[:, ds(kh * 128, 128)], ident[:])
                    hs = hstage.tile([128, 128], ddt, tag="hs", name="hs")
                    nc.scalar.copy(hs[:], pst[:])
                    nc.scalar.dma_start(ht_dram[kh, :, ds(m * 128, 128)], hs[:])

        # ---------------- phase 3: decode ----------------
        # m-pair-major: W_dec is streamed twice, but decode of (m0,m1) only
        # depends on their top-k, so it overlaps top-k of (m2,m3).
        with (
            tc.tile_pool(name="wdp", bufs=4) as wdp,
            tc.tile_pool(name="htp", bufs=4) as htp,
            tc.tile_pool(name="dpsum", bufs=8, space="PSUM") as dps,
            tc.tile_pool(name="devac", bufs=6) as devac,
            tc.tile_pool(name="dbias", bufs=2) as dbias,
        ):
            NT = D // 512
            for mp in range(max(MB // 2, 1)):
                ms = [mp * 2, mp * 2 + 1] if MB > 1 else [0]
                for ng in range(NT // 4):
                    pss = {
                        (m, j): dps.tile(
                            [128, 512], f32, name=f"dps_{m}_{j}", tag="dps"
                        )
                        for m in ms
                        for j in range(4)
                    }
                    bdec_t = dbias.tile([128, 4, 512], f32, tag="bdec", name="bdec_t")
                    nc.scalar.dma_start(
                        bdec_t[:],
                        bdec_p[ds(ng * 2048, 2048)]
                        .rearrange("(j n) -> j n", n=512)
                        .partition_broadcast(128),
                    )
                    for khb in range(KH // 4):
                        hts = {}
                        for m in ms:
                            ht_t = htp.tile(
                                [128, 4, 128], ddt, tag=f"ht{m % 2}", name=f"ht_t{m % 2}"
                            )
                            nc.gpsimd.dma_start(
                                ht_t[:],
                                ht_dram[
                                    ds(khb * 4, 4), :, ds(m * 128, 128)
                                ].rearrange("k p b -> p k b"),
                            )
                            hts[m] = ht_t
                        for k4 in range(4):
                            kh = khb * 4 + k4
                            wd_t = wdp.tile([128, 4, 512], ddt, tag="wd", name="wd_t")
                            nc.sync.dma_start(
                                wd_t[:],
                                wd_p[ds(kh * 128, 128), ds(ng * 2048, 2048)].rearrange(
                                    "p (j n) -> p j n", n=512
                                ),
                            )
                            first = kh == 0
                            last = kh == KH - 1
                            for m in ms:
                                for j in range(4):
                                    nc.tensor.matmul(
                                        pss[(m, j)][:],
                                        hts[m][:, k4],
                                        wd_t[:, j],
                                        start=first,
                                        stop=last,
                                    )
                    for m in ms:
                        for j in range(4):
                            st = devac.tile([128, 512], f32, tag="dst", name="dst")
                            nc.vector.tensor_add(st[:], pss[(m, j)][:], bdec_t[:, j])
                            nc.scalar.dma_start(
                                out_p[
                                    ds(m * 128, 128), ds(ng * 2048 + j * 512, 512)
                                ],
                                st[:],
                            )

    nc.compile()
    return nc


def prepare_inputs(X, W_enc, W_dec, b_enc, b_dec, n_cores=N_CORES):
    """Host-side dtype splits + per-core sharding. X: [B, D]."""
    B, D = X.shape
    H = W_enc.shape[1]
    BC = B // n_cores

    Wr = round_fp32r(W_enc)
    Wb = _bf16(W_enc)
    dWb = _bf16(W_enc - Wr)
    Wdb = _bf16(W_dec)
    benc = np.ascontiguousarray(b_enc, np.float32)
    bdec = np.ascontiguousarray(b_dec, np.float32).reshape(D)

    in_maps = []
    for c in range(n_cores):
        XT = np.ascontiguousarray(X[c * BC : (c + 1) * BC].T)  # [D, BC]
        XTr = round_fp32r(XT)
        in_maps.append(
            {
                "xtr": XTr,
                "xtb": _bf16(XT),
                "dxtb": _bf16(XT - XTr),
                "wr": Wr,
                "wb": Wb,
                "dwb": dWb,
                "wd": Wdb,
                "benc": benc,
                "bdec": bdec,
            }
        )
    return in_maps


_NC_CACHE = {}


def _get_nc(**kw):
    key = tuple(sorted(kw.items()))
    if key not in _NC_CACHE:
        _NC_CACHE[key] = build_nc(**kw)
    return _NC_CACHE[key]


def kernel(activation_BMLD, W_enc_MLDH, W_dec_HMLD, b_enc_H, b_dec_MLD, k, **run_kw):
    assert int(k) == _TOPK
    B = activation_BMLD.shape[0]
    X = np.ascontiguousarray(activation_BMLD, np.float32).reshape(B, _D)
    W_enc = np.ascontiguousarray(W_enc_MLDH, np.float32).reshape(_D, _H)
    W_dec = np.ascontiguousarray(W_dec_HMLD, np.float32).reshape(_H, _D)

    nc = _get_nc(BC=B // N_CORES)
    in_maps = prepare_inputs(X, W_enc, W_dec, b_enc_H, b_dec_MLD)
    res = run_bass_kernel_spmd(nc, in_maps, core_ids=list(range(N_CORES)), **run_kw)
    out = np.concatenate([res.results[c]["out"] for c in range(N_CORES)], axis=0)
    if run_kw.get("trace"):
        kernel.last_result = res
    return out.reshape(B, _M, _L, _DM).astype(np.float32)
